# revision 8
# baseline (speedup 1.0000x reference)
"""PerJointHead Trainium2 Bass kernel.

Computes, for x [B,T,J,D]:
    xn = LayerNorm(x) * gamma + beta          (per (b,t,j) row over D)
    h  = gelu(xn @ fc1_w[j] + fc1_b[j])       (exact erf gelu)
    y  = h @ fc2_w[j] + fc2_b[j]              -> [B,T,J,3]

Sharding: data-parallel over B across 8 cores (4 B each -> 972 tokens per
joint per core).  Weights replicated.  gamma/beta are folded into
fc1_w/fc1_b on the host; x is host-padded to 1024 tokens per core so every
SBUF block is a full 128 partitions and each joint needs one input DMA.

Per-core device pipeline (per joint j):
  1. DMA x[:, j, :] (1024x512) into SBUF natural layout [128 tok, 8 blk, 512].
  2. bn_stats/bn_aggr per 128-token block -> mean/var per row (DVE).
  3. rstd = rsqrt(var+eps) via bit-hack + 3 Newton iterations (DVE only; the
     ACT Sqrt table would thrash with the Gelu table).
  4. xn = (x - mu) * rstd on DVE (tensor_scalar, per-partition scalars).
  5. x^T via PE transpose-mode (fp32 DMA transpose doesn't exist); PSUM->SBUF
     copy on ACT (rounds to fp32r) -> xt [128 d, 4 chunks, 1024 tok].
  6. fc1: out[h-chunk 128, tokens 486] = sum_c W1[c,hc].T @ xt[c]; fp32r
     matmuls (1 cycle/row at N>=256, vs 4 for fp32).
  7. gelu + fc1_b bias in one ACT pass per h-chunk (bias is per-partition in
     the [h, tokens] layout), output rounded to fp32r.
  8. fc2: out[3, tokens] accumulated over 8 h-chunks of h_act; + b2 (DVE).
  9. y^T [3, tok] -> [tok, 3] via PE transpose; gather all joints into
     ynat [128 tok, 8 blk, 17, 3]; contiguous DMA to DRAM at the end.

Engine budget per core (predicted): PE ~330 us (fc1 528k cyc + transposes +
fc2 132k cyc @ 2.4 GHz warm), DVE ~190 us, ACT ~240 us, DMA ~190 us -- PE
bound.  Every instruction is constructed to have at most 2 cross-engine wait
sources (codegen limit: "Too many sync wait commands").
"""

import os
import sys

if "/opt/trn_rl_repo" not in sys.path:
    sys.path.insert(0, "/opt/trn_rl_repo")

import numpy as np

# ---- problem constants (hardcoded per contract) ----
B, T, J, D = 32, 243, 17, 512
H = 2 * D                      # 1024
O = 3
NCORES = 8
BC = B // NCORES               # 4 batches per core
NTOK = BC * T                  # 972 valid tokens per joint per core
P = 128
DC = D // P                    # 4 contraction chunks
HC = H // P                    # 8 h chunks
NBLK = 8                       # padded token blocks
PTOK = NBLK * P                # 1024 padded tokens
NG = 2                         # moving-operand token groups for fc1/fc2
GT = NTOK // NG                # 486 (>=256 keeps fp32r at 1 cycle/row)
YLASTP = NTOK - (NBLK - 1) * P  # 76 valid tokens in the last output block
EPS = 1e-5
RSQRT_MAGIC_P1 = 0x5F3759E0    # 0x5F3759DF + 1 (magic - x == ~x + magic + 1)

_CACHE: dict = {}


def _build_module(mm_dtype_name: str = "float32r"):
    import concourse.bass as bass
    import concourse.bacc as bacc
    import concourse.tile as tile
    from concourse import mybir
    from concourse.bass import ds
    from concourse.masks import make_identity
    from contextlib import ExitStack

    f32 = mybir.dt.float32
    i32 = mybir.dt.int32
    mmdt = getattr(mybir.dt, mm_dtype_name)
    AF = mybir.ActivationFunctionType
    ALU = mybir.AluOpType

    nc = bacc.Bacc("TRN2", target_bir_lowering=False, debug=False,
                   num_devices=NCORES, enable_asserts=False)

    x_d = nc.dram_tensor("x", [PTOK, J, D], f32, kind="ExternalInput").ap()
    w1_d = nc.dram_tensor("w1", [J, P, DC, H], mmdt, kind="ExternalInput").ap()
    b1_d = nc.dram_tensor("b1", [P, J, HC], f32, kind="ExternalInput").ap()
    w2_d = nc.dram_tensor("w2", [P, J, HC, O], mmdt, kind="ExternalInput").ap()
    b2_d = nc.dram_tensor("b2", [O, J], f32, kind="ExternalInput").ap()
    y_d = nc.dram_tensor("y", [NTOK, J, O], f32, kind="ExternalOutput").ap()

    with tile.TileContext(nc) as tc, ExitStack() as ctx:
        singles = ctx.enter_context(tc.tile_pool(name="singles", bufs=1))
        xpool = ctx.enter_context(tc.tile_pool(name="xpool", bufs=2))
        wpool = ctx.enter_context(tc.tile_pool(name="wpool", bufs=2))
        xtpool = ctx.enter_context(tc.tile_pool(name="xtpool", bufs=2))
        hpool = ctx.enter_context(tc.tile_pool(name="hpool", bufs=2))
        spool = ctx.enter_context(tc.tile_pool(name="spool", bufs=2))
        ypool = ctx.enter_context(tc.tile_pool(name="ypool", bufs=2))
        psA = ctx.enter_context(tc.tile_pool(name="psA", bufs=2, space="PSUM"))
        psB = ctx.enter_context(tc.tile_pool(name="psB", bufs=2, space="PSUM"))
        psC = ctx.enter_context(tc.tile_pool(name="psC", bufs=2, space="PSUM"))

        ident = singles.tile([P, P], f32)
        make_identity(nc, ident)
        w2_sb = singles.tile([P, J, HC, O], mmdt)
        nc.sync.dma_start(out=w2_sb, in_=w2_d)
        b1_sb = singles.tile([P, J, HC], f32)
        nc.sync.dma_start(out=b1_sb, in_=b1_d)
        b2_sb = singles.tile([O, J], f32)
        nc.sync.dma_start(out=b2_sb, in_=b2_d)
        # whole-core output staging: [128 tok, blk, joint, 3]
        ynat = singles.tile([P, NBLK, J, O], f32)

        for j in range(J):
            w1_sb = wpool.tile([P, DC, H], mmdt, tag="w1")
            nc.sync.dma_start(out=w1_sb, in_=w1_d[j])

            xnat = xpool.tile([P, NBLK, D], f32, tag="xnat")
            nc.sync.dma_start(
                out=xnat,
                in_=x_d[:, j, :].rearrange("(b p) d -> p b d", p=P),
            )

            stats = spool.tile([P, NBLK, 6], f32, tag="stats")
            mv = spool.tile([P, NBLK, 2], f32, tag="mv")
            for b in range(NBLK):
                nc.vector.bn_stats(out=stats[:, b, :], in_=xnat[:, b, :])
                nc.vector.bn_aggr(out=mv[:, b, :], in_=stats[:, b, :])

            # rstd = rsqrt(var + eps): bit-hack seed + 3 Newton steps (DVE only)
            vv = spool.tile([P, NBLK], f32, tag="vv")
            nc.vector.tensor_scalar_add(vv, mv[:, :, 1], EPS)
            yi = spool.tile([P, NBLK], i32, tag="yi")
            nc.vector.tensor_scalar(
                out=yi, in0=vv.bitcast(i32), scalar1=1, scalar2=-1,
                op0=ALU.logical_shift_right, op1=ALU.bitwise_xor)
            nc.vector.tensor_scalar_add(yi, yi, RSQRT_MAGIC_P1)
            rstd = yi.bitcast(f32)
            t0 = spool.tile([P, NBLK], f32, tag="t0")
            for _ in range(3):
                nc.vector.tensor_mul(t0, rstd, rstd)
                nc.vector.tensor_mul(t0, t0, vv)
                nc.vector.tensor_scalar(out=t0, in0=t0, scalar1=-0.5,
                                        scalar2=1.5, op0=ALU.mult, op1=ALU.add)
                nc.vector.tensor_mul(rstd, rstd, t0)

            # normalize on DVE: xn = (x - mu) * rstd
            xn = xpool.tile([P, NBLK, D], f32, tag="xn")
            for b in range(NBLK):
                nc.vector.tensor_scalar(
                    out=xn[:, b, :], in0=xnat[:, b, :],
                    scalar1=mv[:, b, 0:1], scalar2=rstd[:, b:b + 1],
                    op0=ALU.subtract, op1=ALU.mult)

            # PE transpose -> xt [128 d, DC, PTOK]; PSUM->SBUF copy on ACT
            xt = xtpool.tile([P, DC, PTOK], mmdt, tag="xt")
            for b in range(NBLK):
                pst = psA.tile([P, DC, P], f32, tag="pst")
                for c in range(DC):
                    nc.tensor.transpose(pst[:, c, :],
                                        xn[:, b, ds(c * P, P)],
                                        ident)
                nc.scalar.copy(out=xt[:, :, ds(b * P, P)], in_=pst)

            ytj = ypool.tile([O, NTOK], f32, tag="ytj")
            for g in range(NG):
                hact = hpool.tile([P, HC, GT], mmdt, tag="hact")
                for hc in range(HC):
                    psh = psB.tile([P, GT], f32, tag="psh")
                    for c in range(DC):
                        nc.tensor.matmul(psh,
                                         w1_sb[:, c, ds(hc * P, P)],
                                         xt[:, c, ds(g * GT, GT)],
                                         start=(c == 0), stop=(c == DC - 1))
                    nc.scalar.activation(out=hact[:, hc, :], in_=psh,
                                         func=AF.Gelu,
                                         bias=b1_sb[:, j, hc:hc + 1],
                                         scale=1.0)
                psy = psC.tile([O, GT], f32, tag="psy")
                for hc in range(HC):
                    nc.tensor.matmul(psy,
                                     w2_sb[:, j, hc, :],
                                     hact[:, hc, :],
                                     start=(hc == 0), stop=(hc == HC - 1))
                nc.vector.tensor_scalar_add(ytj[:, ds(g * GT, GT)], psy,
                                            b2_sb[:, j:j + 1])

            # y^T [3, tok] -> [tok, 3] per block, into ynat
            for b in range(NBLK):
                pb = P if b < NBLK - 1 else YLASTP
                psyt = psC.tile([P, O], f32, tag="psyt")
                nc.tensor.transpose(psyt[:pb, :],
                                    ytj[:, ds(b * P, pb)],
                                    ident[:O, :O])
                nc.vector.tensor_copy(out=ynat[:pb, b, j, :], in_=psyt[:pb, :])

        nc.sync.dma_start(
            out=y_d[: (NBLK - 1) * P].rearrange("(b p) j o -> p b j o", p=P),
            in_=ynat[:, : NBLK - 1, :, :],
        )
        nc.sync.dma_start(out=y_d[(NBLK - 1) * P:],
                          in_=ynat[:YLASTP, NBLK - 1, :, :])

    nc.compile()
    return nc


def get_module(mm_dtype_name: str = "float32r"):
    key = ("nc", mm_dtype_name)
    if key not in _CACHE:
        _CACHE[key] = _build_module(mm_dtype_name)
    return _CACHE[key]


def _host_prep(ln_gamma, ln_beta, fc1_w, fc1_b, fc2_w, fc2_b):
    """Fold gamma/beta into fc1; reshape weights to device layouts."""
    ln_gamma = np.asarray(ln_gamma, np.float32)
    ln_beta = np.asarray(ln_beta, np.float32)
    fc1_w = np.asarray(fc1_w, np.float32)
    fc1_b = np.asarray(fc1_b, np.float32)
    fc2_w = np.asarray(fc2_w, np.float32)
    fc2_b = np.asarray(fc2_b, np.float32)

    w1p = ln_gamma[None, :, None] * fc1_w                      # [J, D, H]
    b1p = fc1_b + np.einsum("d,jdh->jh", ln_beta, fc1_w)       # [J, H]

    # lhsT layout per joint: [128 (d within chunk), DC, H]
    w1_dev = np.ascontiguousarray(
        w1p.reshape(J, DC, P, H).transpose(0, 2, 1, 3))        # [J,128,DC,H]
    b1_dev = np.ascontiguousarray(
        b1p.reshape(J, HC, P).transpose(2, 0, 1))              # [128,J,HC]
    w2_dev = np.ascontiguousarray(
        fc2_w.reshape(J, HC, P, O).transpose(2, 0, 1, 3))      # [128,J,HC,O]
    b2_dev = np.ascontiguousarray(fc2_b.T)                     # [O,J]
    return w1_dev, b1_dev, w2_dev, b2_dev


def kernel(x, ln_gamma, ln_beta, fc1_w, fc1_b, fc2_w, fc2_b):
    from concourse.bass_utils import run_bass_kernel_spmd

    x = np.asarray(x, np.float32)
    w1_dev, b1_dev, w2_dev, b2_dev = _host_prep(
        ln_gamma, ln_beta, fc1_w, fc1_b, fc2_w, fc2_b)

    nc = get_module(os.environ.get("PJH_MM_DTYPE", "float32r"))

    in_maps = []
    for c in range(NCORES):
        xc = np.zeros((PTOK, J, D), np.float32)
        xc[:NTOK] = x[c * BC:(c + 1) * BC].reshape(NTOK, J, D)
        in_maps.append({"x": xc, "w1": w1_dev, "b1": b1_dev,
                        "w2": w2_dev, "b2": b2_dev})

    trace = os.environ.get("PJH_TRACE", "0") == "1"
    res = run_bass_kernel_spmd(nc, in_maps, core_ids=list(range(NCORES)),
                               trace=trace)
    _CACHE["last_results"] = res

    y = np.concatenate(
        [r["y"].reshape(BC, T, J, O) for r in res.results], axis=0)
    return y
